# revision 7
# baseline (speedup 1.0000x reference)
"""Variable-length average pooling (prefix mean over seq axis) on 8 trn2 cores.

Strategy (pure data parallelism over batch, packed row stream):
  - eff_len[b] = lengths[b] if >0 else L.  pooled[b] = sum_{l<eff} x[b,l,:] / eff.
  - Sort batches by eff_len desc, snake-assign 16 per core; on the host, pack
    ONLY the valid rows of each core's 16 batches into one contiguous stream
    [R_c, 2048], zero-padded to NKC*128 rows (NKC = max_c ceil(R_c/128)).
    No per-slot round-up waste: 68 chunks/core here vs 78 for chunk-per-slot
    (-13% DMA bytes, the memory-bound floor).
  - One SPMD Bass program shared by all 8 cores; per-core raggedness lives
    entirely in a host-built weight tensor.  The two compute engines split
    the chunk stream so neither can throttle the DMA flow control:
      * chunk k whose 128 rows belong to ONE slot on EVERY core ("interior",
        ~2/3 of chunks) is summed on the VectorE into an SBUF accumulator
        (fp32 adds, ~2.2us each), later folded into PSUM by one matmul with
        a [128,16] column-select weight (col s = 1/eff_s);
      * every other chunk ("mixed": slot boundaries, tiny tail slots, pad)
        goes to the PE as psum[16,512] += wmat_k[128,16].T @ tile[128,512],
        wmat_k[p,s] = (row p owned by slot s on this core)/eff_s, pad rows 0.
    All matmuls accumulate into a single PSUM region [16,2048] (4 banks, one
    accumulation group per bank): no per-slot PSUM recycling, and a small
    tail (last reduce -> one [16,2048] PSUM->SBUF copy -> one 128 KB DMA).
    A PE-only variant measures ~5.7us per 2 MB pair on the matmul stream
    (burst-gap pattern), which throttles the loads; the DVE split keeps
    both consumers at ~half the DMA rate.
  - Tensors/tiles are declared float32r (same bytes as fp32): single-pass
    fp32 on the PE at ~1 cycle/row for free dim 512, vs 4 cycles/row for
    exact fp32 (2 half-rate passes).  Measured rel err ~1.4e-4 (PSUM still
    accumulates fp32) vs the 2e-2 gate.  A .bitcast() view fails walrus
    codegen; native dtype works.
  - Chunks are fetched full-width (128 rows) in 2 MB pairs, alternating the
    two HWDGE rings (SP/ACT), which sustain ~410 GB/s together when the
    consumers keep up; a third SWDGE ring (Pool) measures WORSE (~324 GB/s
    aggregate - more engine time per byte).
"""

import os

import numpy as np

import concourse.bacc as bacc
import concourse.mybir as mybir
from concourse.tile import TileContext
from concourse.bass_utils import run_bass_kernel_spmd

B, L, D = 128, 1024, 2048
NCORES = 8
SLOTS = B // NCORES  # 16
P = 128              # rows per chunk (partition dim)
NTILE = 512          # matmul free dim (one PSUM bank of fp32)

TILE_BUFS = int(os.environ.get("TILE_BUFS", "10"))
FP32R = os.environ.get("FP32R", "1") == "1"

LAST_RESULTS = None  # BassKernelResults of the most recent device run


def _plan(eff):
    """Snake-assign sorted batches to cores and derive the shared chunk plan.

    Returns (cores[c][s] -> batch idx, offs[c][s] -> packed row offset,
    owner[c][r] -> slot owning packed row r (-1 pad), plan) where plan is
    the core-independent program structure."""
    order = np.argsort(-eff, kind="stable")
    cores = [[] for _ in range(NCORES)]
    for i, idx in enumerate(order):
        blk, pos = divmod(i, NCORES)
        c = pos if blk % 2 == 0 else NCORES - 1 - pos
        cores[c].append(int(idx))
    offs = np.zeros((NCORES, SLOTS + 1), dtype=np.int64)
    for c in range(NCORES):
        offs[c, 1:] = np.cumsum([eff[b] for b in cores[c]])
    NKC = int(-(-offs[:, -1].max() // P))
    owner = np.full((NCORES, NKC * P), -1, dtype=np.int32)
    for c in range(NCORES):
        for s in range(SLOTS):
            owner[c, offs[c, s] : offs[c, s + 1]] = s

    chunk_class = []  # per k: ('dve', s) or ('pe', None)
    interior = {}     # slot -> list of interior chunk ks
    for k in range(NKC):
        u = np.unique(owner[:, k * P : (k + 1) * P])
        if len(u) == 1 and u[0] >= 0:
            s = int(u[0])
            chunk_class.append(("dve", s))
            interior.setdefault(s, []).append(k)
        else:
            chunk_class.append(("pe", None))

    n_pe = 0
    pe_idx = {}
    for k, (t, _) in enumerate(chunk_class):
        if t == "pe":
            pe_idx[k] = n_pe
            n_pe += 1
    reduce_after = {}  # chunk k -> slots whose acc-reduce is emitted after k
    reduce_idx = {}
    n_red = 0
    for k in range(NKC):
        for s, ks in interior.items():
            if ks[-1] == k:
                reduce_after.setdefault(k, []).append(s)
                reduce_idx[s] = n_red
                n_red += 1
    plan = (
        NKC,
        tuple(chunk_class),
        {s: tuple(v) for s, v in interior.items()},
        {k: tuple(v) for k, v in reduce_after.items()},
        pe_idx,
        reduce_idx,
        n_pe,
        n_red,
    )
    return cores, offs, owner, plan


def _plan_cache_key(plan):
    NKC, chunk_class, interior, reduce_after, _, _, n_pe, n_red = plan
    return (
        NKC,
        chunk_class,
        tuple(sorted(interior.items())),
        tuple(sorted(reduce_after.items())),
        n_pe,
        n_red,
        TILE_BUFS,
        FP32R,
    )


_PROGRAM_CACHE = {}


def _build_program(plan):
    NKC, chunk_class, interior, reduce_after, pe_idx, reduce_idx, n_pe, n_red = plan
    WCOLS = SLOTS * (n_pe + n_red)
    first_interior = {s: ks[0] for s, ks in interior.items()}
    n_groups = n_pe + n_red  # matmul accumulation contributions per bank

    # Bacc (not raw Bass): its compile pass splits multi-sem waits and moves
    # matmul waits onto ldweights — walrus allows only 1 wait per instruction.
    nc = bacc.Bacc(None, target_bir_lowering=False)
    f32 = mybir.dt.float32
    td = mybir.dt.float32r if FP32R else f32
    feat = nc.dram_tensor("features", [NKC * P, D], td, kind="ExternalInput")
    wmat = nc.dram_tensor("wmat", [P, WCOLS], td, kind="ExternalInput")
    out = nc.dram_tensor("out", [SLOTS, D], f32, kind="ExternalOutput")

    with TileContext(nc) as tc:
        with (
            tc.tile_pool(name="w", bufs=1) as wpool,
            tc.tile_pool(name="tiles", bufs=TILE_BUFS) as tpool,
            tc.tile_pool(name="accs", bufs=2) as apool,
            tc.tile_pool(name="psum", bufs=1, space="PSUM") as ppool,
            tc.tile_pool(name="outs", bufs=1) as opool,
        ):
            dma_engines = [nc.sync, nc.scalar]
            w_tile = wpool.tile([P, WCOLS], td)
            dma_engines[-1].dma_start(out=w_tile[:], in_=wmat[:])
            psum_t = ppool.tile([SLOTS, D], f32)

            # Load chunks in 2 MB pairs [128, 2D] (chunk halves side by side),
            # alternating the two rings; odd leftover as a 1 MB single.
            halves = {}  # chunk k -> (tile, col offset)
            n_dma = 0
            k = 0
            while k < NKC:
                if k + 1 < NKC:
                    pair = tpool.tile([P, 2 * D], td, name="pair", tag="t")
                    src = feat[k * P : (k + 2) * P, :].rearrange(
                        "(c p) d -> p c d", p=P
                    )
                    dst = pair[:].rearrange("p (c d) -> p c d", c=2)
                    dma_engines[n_dma % 2].dma_start(out=dst, in_=src)
                    halves[k] = (pair, 0)
                    halves[k + 1] = (pair, D)
                    k += 2
                else:
                    single = tpool.tile([P, D], td, name="single", tag="t")
                    dma_engines[n_dma % 2].dma_start(
                        out=single[:], in_=feat[k * P : (k + 1) * P, :]
                    )
                    halves[k] = (single, 0)
                    k += 1
                n_dma += 1

            accs = {}
            g = 0  # accumulation-group contribution counter (per bank)

            def mm(wcol_off, rhs_tile, rhs_off):
                nonlocal g
                for j in range(D // NTILE):
                    nc.tensor.matmul(
                        psum_t[0:SLOTS, j * NTILE : (j + 1) * NTILE],
                        w_tile[:, wcol_off : wcol_off + SLOTS],
                        rhs_tile[:, rhs_off + j * NTILE : rhs_off + (j + 1) * NTILE],
                        start=(g == 0),
                        stop=(g == n_groups - 1),
                    )
                g += 1

            for k in range(NKC):
                t, s = chunk_class[k]
                tile, off = halves[k]
                if t == "dve":
                    if k == first_interior[s]:
                        accs[s] = apool.tile([P, D], td, name="acc", tag="acc")
                        nc.vector.tensor_copy(
                            out=accs[s][:], in_=tile[:, off : off + D]
                        )
                    else:
                        nc.vector.tensor_add(
                            out=accs[s][:], in0=accs[s][:], in1=tile[:, off : off + D]
                        )
                else:
                    mm(SLOTS * pe_idx[k], tile, off)
                for s2 in reduce_after.get(k, ()):
                    mm(SLOTS * (n_pe + reduce_idx[s2]), accs[s2], 0)

            # DVE for the PSUM->SBUF copy: the DMA-issuing sequencers are
            # busy draining loads.
            out_t = opool.tile([SLOTS, D], f32)
            nc.vector.tensor_copy(out=out_t[:], in_=psum_t[:])
            dma_engines[-1].dma_start(out=out[:], in_=out_t[:])
    nc.finalize()
    return nc


def kernel(features, lengths):
    global LAST_RESULTS
    features = np.ascontiguousarray(features, dtype=np.float32)
    lengths = np.ascontiguousarray(lengths, dtype=np.int32)
    eff = np.where(lengths > 0, lengths, L).astype(np.int64)

    cores, offs, owner, plan = _plan(eff)
    NKC, chunk_class, interior, reduce_after, pe_idx, reduce_idx, n_pe, n_red = plan
    WCOLS = SLOTS * (n_pe + n_red)

    key = _plan_cache_key(plan)
    if key not in _PROGRAM_CACHE:
        _PROGRAM_CACHE[key] = _build_program(plan)
    nc = _PROGRAM_CACHE[key]

    in_maps = []
    for c in range(NCORES):
        perm = cores[c]
        inv = 1.0 / eff[perm].astype(np.float32)
        packed = np.zeros((NKC * P, D), dtype=np.float32)
        for s, b in enumerate(perm):
            packed[offs[c, s] : offs[c, s + 1]] = features[b, : eff[b]]
        wmat = np.zeros((P, WCOLS), dtype=np.float32)
        own_c = owner[c].reshape(NKC, P)
        for k, (t, _) in enumerate(chunk_class):
            if t == "pe":
                o = own_c[k]
                valid = o >= 0
                wmat[valid, SLOTS * pe_idx[k] + o[valid]] = inv[o[valid]]
        for s, ridx in reduce_idx.items():
            wmat[:, SLOTS * (n_pe + ridx) + s] = inv[s]
        in_maps.append({"features": packed, "wmat": wmat})

    trace = os.environ.get("KERNEL_TRACE", "0") == "1"
    LAST_RESULTS = run_bass_kernel_spmd(
        nc,
        in_maps,
        core_ids=list(range(NCORES)),
        trace=trace,
        trace_cores=[0] if trace else None,
    )

    out = np.empty((B, D), dtype=np.float32)
    for c in range(NCORES):
        out[np.asarray(cores[c])] = LAST_RESULTS.results[c]["out"]
    return out
